# revision 13
# baseline (speedup 1.0000x reference)
"""Belief-propagation single-iteration kernel for 8 Trainium2 NeuronCores.

Problem (see reference):
    theta0: (2048, 8192) f32   clique A over (v0, v1_fine)
    theta1: (4096, 2048) f32   clique B over (v1_coarse, v2)
    idx_a, idx_b: (8192,) int64 maps fine->coarse

    marg_a = logsumexp(theta0, axis=0)                      # (8192,)
    msg_ab = segment_logsumexp(marg_a, idx_b, 4096)         # (4096,)
    marg_b = logsumexp(theta1, axis=1)                      # (4096,)
    msg_ba = marg_b[idx_a]                                  # (8192,)
    t0 = theta0 + msg_ba[None, :];  t0 -= logsumexp(t0)
    t1 = theta1 + msg_ab[:, None];  t1 -= logsumexp(t1)
    return (t0, t1)

Device algorithm works in the linear "sum-exp" domain:
    S_a[j] = sum_i exp(theta0[i,j])        (AllReduce over row shards)
    S_b[c] = sum_k exp(theta1[c,k])        (AllReduce over col shards)
    seg[c] = sum_{j: idx_b[j]=c} S_a[j]    (CSR-padded ap_gather + reduce)
    dot0 = sum_j S_a[j] * S_b[idx_a[j]]  = exp(Z0)
    dot1 = sum_c S_b[c] * seg[c]         = exp(Z1)
    t0 = theta0 + ln(S_b[idx_a[j]] / dot0)   (broadcast add over rows)
    t1 = theta1 + ln(seg[c] / dot1)          (broadcast add over cols)

Sharding: theta0 row-sharded (v0), theta1 column-sharded (v2); the small
marginal vectors are AllReduced and the message computation runs
replicated on every core.  Fully SPMD - no core-id dependence anywhere.

The S_b table is stored in a "sigma" permuted order sigma(c) =
(c % 128) * 32 + c // 128 so that the per-core [128, 32] partial-sum
tile DMAs to the collective buffer with a plain access pattern; idx_a
is remapped on the host to compensate.
"""

import numpy as np

D0, D1F, D1C, D2 = 2048, 8192, 4096, 2048
NCORES = 8
R0 = D0 // NCORES  # theta0 rows per core (256)
C2 = D2 // NCORES  # theta1 cols per core (256)
T0T = R0 // 128  # theta0 SBUF tiles per core (2)
T1T = D1C // 128  # theta1 row tiles (32)
NCHUNK = D1F // 512  # 512-wide chunks of the fine axis (16)
PAD = 128  # zero slots appended to the S_a gather table

_cache = {}


def _build(cap, repeats=1, no_gather=False, stage=99):
    import concourse.bacc as bacc
    import concourse.bass as bass
    import concourse.tile as tile
    from concourse import mybir

    f32 = mybir.dt.float32
    i16 = mybir.dt.int16
    Exp = mybir.ActivationFunctionType.Exp
    Ln = mybir.ActivationFunctionType.Ln
    X = mybir.AxisListType.X
    add = mybir.AluOpType.add
    mult = mybir.AluOpType.mult

    nc = bacc.Bacc("TRN2", target_bir_lowering=False, debug=False, num_devices=NCORES)

    th0 = nc.dram_tensor("th0", [R0, D1F], f32, kind="ExternalInput").ap()
    th1 = nc.dram_tensor("th1", [D1C, C2], f32, kind="ExternalInput").ap()
    idxa = nc.dram_tensor("idxa", [128, D1F // 16], i16, kind="ExternalInput").ap()
    idxb = nc.dram_tensor("idxb", [128, D1C // 8 * cap // 16], i16,
                          kind="ExternalInput").ap()
    t0 = nc.dram_tensor("t0", [R0, D1F], f32, kind="ExternalOutput").ap()
    t1 = nc.dram_tensor("t1", [D1C, C2], f32, kind="ExternalOutput").ap()

    nseg = 512 * cap  # per-core gather positions for the segment sum

    with tile.TileContext(nc) as tc:
        with (
            tc.tile_pool(name="singles", bufs=1) as singles,
            tc.tile_pool(name="big", bufs=1) as big,
            tc.tile_pool(name="stage", bufs=3) as stg,
            tc.tile_pool(name="th1s", bufs=6) as th1s,
            tc.tile_pool(name="psum", bufs=4, space="PSUM") as psum,
            tc.tile_pool(name="dram", bufs=1, space="DRAM") as dram,
        ):
            ones = singles.tile([128, 128], f32)
            nc.vector.memset(ones, 1.0)
            idxa_sb = singles.tile([128, D1F // 16], i16)
            nc.sync.dma_start(out=idxa_sb, in_=idxa)
            idxb_sb = singles.tile([128, D1C // 8 * cap // 16], i16)
            nc.sync.dma_start(out=idxb_sb, in_=idxb)

            for _rep in range(repeats):
                # Resident theta0 shard: [128, t, 8192], t in {0, 1}
                th0_sb = big.tile([128, T0T, D1F], f32, tag="th0")
                sa_tbl = big.tile([128, D1F + PAD], f32, tag="sa")
                sbg = big.tile([128, D1F], f32, tag="sbg")
                sb_tbl = big.tile([128, D1C], f32, tag="sbt")
                segout = big.tile([128, nseg], f32, tag="seg")
                sb_col = singles.tile([128, T1T], f32, tag="sbcol")
                sbl_col = singles.tile([128, T1T], f32, tag="sblcol")
                prod1 = singles.tile([128, T1T], f32, tag="prod1")
                red = singles.tile([128, 512], f32, tag="red")
                dot0 = singles.tile([128, 1], f32, tag="dot0")
                recip0 = singles.tile([128, 1], f32, tag="rec0")
                recip1 = singles.tile([128, 1], f32, tag="rec1")
                pr_col = singles.tile([128, 1], f32, tag="prc")
                mg_lin = singles.tile([128, T1T], f32, tag="mgl")
                mg_col = singles.tile([128, T1T], f32, tag="mgc")

                cc_in = dram.tile([1, D1F + D1C], f32, tag="ccin")
                cc_out = dram.tile([1, D1F + D1C], f32, tag="ccout")
                msg_s = dram.tile([1, D1C], f32, tag="msgs")

                nc.vector.memset(sa_tbl[:, D1F:], 0.0)

                # ---- phase A: theta1 pass 1 -> S_b partials --------------
                for t in range(T1T):
                    t1_in = stg.tile([128, C2], f32, tag="t1a")
                    nc.sync.dma_start(out=t1_in, in_=th1[128 * t:128 * (t + 1), :])
                    e1 = stg.tile([128, C2], f32, tag="t1e")
                    nc.scalar.activation(e1, t1_in, Exp)
                    nc.vector.tensor_reduce(sb_col[:, t:t + 1], e1, axis=X, op=add)
                # natural [p, t] layout == sigma order sigma(c)=(c%128)*32+c//128
                nc.sync.dma_start(out=cc_in[0, D1F:], in_=sb_col)

                if stage < 2:
                    continue
                # ---- phase B: theta0 stream in -> S_a partials -----------
                for n in range(NCHUNK):
                    sl = slice(512 * n, 512 * (n + 1))
                    for t in range(T0T):
                        nc.sync.dma_start(
                            out=th0_sb[:, t, sl],
                            in_=th0[128 * t:128 * (t + 1), sl])
                    e0 = stg.tile([128, 512], f32, tag="e0")
                    e1b = stg.tile([128, 512], f32, tag="e1b")
                    nc.scalar.activation(e0, th0_sb[:, 0, sl], Exp)
                    nc.scalar.activation(e1b, th0_sb[:, 1, sl], Exp)
                    nc.vector.tensor_add(e0, e0, e1b)
                    ps = psum.tile([128, 512], f32, tag="ps")
                    nc.tensor.matmul(ps, ones, e0, start=True, stop=True)
                    # every PSUM row holds the same column sums; ship row 0
                    sa_row = stg.tile([1, 512], f32, tag="sarow")
                    nc.vector.tensor_copy(sa_row, ps[0:1, :])
                    nc.sync.dma_start(out=cc_in[0, sl], in_=sa_row)

                if stage < 3:
                    continue
                # ---- AllReduce -------------------------------------------
                nc.gpsimd.collective_compute(
                    "AllReduce",
                    mybir.AluOpType.add,
                    replica_groups=[list(range(NCORES))],
                    ins=[cc_in.opt()],
                    outs=[cc_out.opt()],
                )

                if stage < 4:
                    continue
                # ---- broadcast global tables to all partitions -----------
                nc.sync.dma_start(
                    out=sa_tbl[:, :D1F],
                    in_=cc_out[0, :D1F].partition_broadcast(128))
                nc.sync.dma_start(
                    out=sb_tbl, in_=cc_out[0, D1F:].partition_broadcast(128))
                # global S_b in [p, t] layout (sigma order is exactly p*32+t)
                nc.sync.dma_start(
                    out=sbl_col,
                    in_=cc_out[0, D1F:].rearrange("(p t) -> p t", t=T1T))

                if stage < 5:
                    continue
                # ---- gathers ---------------------------------------------
                if no_gather:
                    nc.vector.memset(sbg, 1.0)
                    nc.vector.memset(segout, 1.0)
                else:
                    # msg_ba (bcast over all partitions): S_b[sigma(idx_a[j])]
                    nc.gpsimd.ap_gather(
                        out_ap=sbg[:].rearrange("p (n d) -> p n d", d=1),
                        in_ap=sb_tbl[:].rearrange("p (n d) -> p n d", d=1),
                        idxs_ap=idxa_sb[:],
                        channels=128, num_elems=D1C, d=1, num_idxs=D1F)
                    # segment sums: core k's partition group holds buckets
                    # [512k, 512k+512) x cap CSR slots
                    nc.gpsimd.ap_gather(
                        out_ap=segout[:].rearrange("p (n d) -> p n d", d=1),
                        in_ap=sa_tbl[:].rearrange("p (n d) -> p n d", d=1),
                        idxs_ap=idxb_sb[:],
                        channels=128, num_elems=D1F + PAD, d=1, num_idxs=nseg)
                nc.vector.tensor_reduce(
                    red, segout[:].rearrange("p (b l) -> p b l", l=cap),
                    axis=X, op=add)

                if stage < 6:
                    continue
                # ---- msg_ab assembly (c-linear via DRAM bounce) ----------
                for k in range(8):
                    nc.sync.dma_start(out=msg_s[0, 512 * k:512 * (k + 1)],
                                      in_=red[16 * k:16 * k + 1, :])
                m0 = msg_s[0, :]
                ml_src = bass.AP(tensor=m0.tensor, offset=m0.offset,
                                 ap=[[1, 128], [128, T1T]])
                nc.sync.dma_start(out=mg_lin, in_=ml_src)

                if stage < 62:
                    continue
                # ---- normalizers -----------------------------------------
                # dot0 = sum_j S_a[j] * S_b[idx_a[j]]  (same on every partition)
                nc.vector.tensor_mul(sa_tbl[:, :D1F], sa_tbl[:, :D1F], sbg)
                nc.vector.tensor_reduce(dot0, sa_tbl[:, :D1F], axis=X, op=add)
                nc.vector.reciprocal(out=recip0, in_=dot0)
                if stage < 63:
                    continue
                # dot1 = sum_c S_b[c] * seg[c]; mg_lin and sbl_col share the
                # (p, t) <-> c = 128t + p layout, so this is one fused
                # multiply-reduce plus a ones-matmul partition sum
                nc.vector.tensor_mul(prod1, mg_lin, sbl_col)
                nc.vector.tensor_reduce(pr_col, prod1, axis=X, op=add)
                pr_ps = psum.tile([128, 1], f32, tag="prps")
                nc.tensor.matmul(pr_ps, ones, pr_col, start=True, stop=True)
                nc.vector.reciprocal(out=recip1, in_=pr_ps)

                if stage < 64:
                    continue
                # ---- log-domain messages ---------------------------------
                # msgba_adj[j] = ln(S_b[idx_a[j]] / dot0), broadcast layout
                nc.scalar.activation(sbg, sbg, Ln, scale=recip0[:, 0:1])
                # msgab_col[p, t] = ln(seg[128t+p] / dot1)
                nc.scalar.activation(mg_col, mg_lin, Ln, scale=recip1[:, 0:1])

                if stage < 7:
                    continue
                # ---- final broadcast adds + writeback --------------------
                for t in range(T0T):
                    for h in range(4):
                        sl = slice(2048 * h, 2048 * (h + 1))
                        nc.vector.tensor_add(th0_sb[:, t, sl], th0_sb[:, t, sl],
                                             sbg[:, sl])
                        nc.sync.dma_start(out=t0[128 * t:128 * (t + 1), sl],
                                          in_=th0_sb[:, t, sl])
                for t in range(T1T):
                    t1_io = th1s.tile([128, C2], f32, tag="t1b")
                    nc.sync.dma_start(out=t1_io, in_=th1[128 * t:128 * (t + 1), :])
                    nc.vector.tensor_scalar_add(t1_io, t1_io, mg_col[:, t:t + 1])
                    nc.sync.dma_start(out=t1[128 * t:128 * (t + 1), :], in_=t1_io)

    nc.compile()
    return nc


def _prep_inputs(theta0, theta1, idx_a, idx_b):
    """Host-side sharding + index-table construction (no float math)."""
    theta0 = np.ascontiguousarray(np.asarray(theta0, dtype=np.float32))
    theta1 = np.ascontiguousarray(np.asarray(theta1, dtype=np.float32))
    ia = np.asarray(idx_a).astype(np.int64)
    ib = np.asarray(idx_b).astype(np.int64)

    # sigma-permuted idx_a (S_b table is stored in sigma order)
    sig_ia = ((ia % 128) * 32 + ia // 128).astype(np.int16)
    idxa_w = np.ascontiguousarray(
        np.tile(sig_ia.reshape(D1F // 16, 16).T, (8, 1)))  # [128, 512]

    # CSR with per-bucket capacity cap, padded with D1F (a zero slot)
    counts = np.bincount(ib, minlength=D1C)
    cap = int(max(2, counts.max()))
    order = np.argsort(ib, kind="stable")
    sc = ib[order]
    ranks = np.arange(D1F) - np.searchsorted(sc, sc, side="left")
    G = np.full((D1C, cap), D1F, dtype=np.int16)
    G[sc, ranks] = order.astype(np.int16)

    idxb_w = np.empty((128, 512 * cap // 16), dtype=np.int16)
    i = np.arange(512 * cap)
    for k in range(8):
        unwrapped = G[512 * k + i // cap, i % cap]
        idxb_w[16 * k:16 * (k + 1), :] = unwrapped.reshape(512 * cap // 16, 16).T

    in_maps = []
    for k in range(NCORES):
        in_maps.append({
            "th0": theta0[R0 * k:R0 * (k + 1), :],
            "th1": np.ascontiguousarray(theta1[:, C2 * k:C2 * (k + 1)]),
            "idxa": idxa_w,
            "idxb": idxb_w,
        })
    return cap, in_maps


def get_program(cap, repeats=1):
    key = ("prog", cap, repeats)
    if key not in _cache:
        _cache[key] = _build(cap, repeats)
    return _cache[key]


def kernel(theta0, theta1, idx_a, idx_b):
    from concourse.bass_utils import run_bass_kernel_spmd

    cap, in_maps = _prep_inputs(theta0, theta1, idx_a, idx_b)
    nc = get_program(cap)
    res = run_bass_kernel_spmd(nc, in_maps, list(range(NCORES)))
    t0 = np.concatenate([res.results[k]["t0"] for k in range(NCORES)], axis=0)
    t1 = np.concatenate([res.results[k]["t1"] for k in range(NCORES)], axis=1)
    return (t0, t1)


# revision 15
# speedup vs baseline: 1.5617x; 1.5617x over previous
"""Belief-propagation single-iteration kernel for 8 Trainium2 NeuronCores.

Problem (see reference):
    theta0: (2048, 8192) f32   clique A over (v0, v1_fine)
    theta1: (4096, 2048) f32   clique B over (v1_coarse, v2)
    idx_a, idx_b: (8192,) int64 maps fine->coarse

    marg_a = logsumexp(theta0, axis=0)                      # (8192,)
    msg_ab = segment_logsumexp(marg_a, idx_b, 4096)         # (4096,)
    marg_b = logsumexp(theta1, axis=1)                      # (4096,)
    msg_ba = marg_b[idx_a]                                  # (8192,)
    t0 = theta0 + msg_ba[None, :];  t0 -= logsumexp(t0)
    t1 = theta1 + msg_ab[:, None];  t1 -= logsumexp(t1)
    return (t0, t1)

Device algorithm works in the linear "sum-exp" domain:
    S_a[j] = sum_i exp(theta0[i,j])        (AllReduce over row shards)
    S_b[c] = sum_k exp(theta1[c,k])        (AllReduce over col shards)
    seg[c] = sum_{j: idx_b[j]=c} S_a[j]    (CSR-padded ap_gather + reduce)
    dot0 = sum_j S_a[j] * S_b[idx_a[j]]  = exp(Z0)
    dot1 = sum_c S_b[c] * seg[c]         = exp(Z1)
    t0 = theta0 + ln(S_b[idx_a[j]] / dot0)   (broadcast add over rows)
    t1 = theta1 + ln(seg[c] / dot1)          (broadcast add over cols)

Sharding: theta0 row-sharded (v0), theta1 column-sharded (v2); the small
marginal vectors are AllReduced and the message computation runs
replicated on every core.  Fully SPMD - no core-id dependence anywhere.

The S_b table is stored in a "sigma" permuted order sigma(c) =
(c % 128) * 32 + c // 128 so that the per-core [128, 32] partial-sum
tile DMAs to the collective buffer with a plain access pattern; idx_a
is remapped on the host to compensate.

Partition-broadcasts of the small tables use K=1 ones-matmuls on the PE
(a 0-stride broadcast DMA measures ~65 GB/s - far too slow).  The
msg_ba gather is split 8 ways across the GPSIMD cores (ap_gather costs
~20 ns per index per core), then reassembled through a DRAM row and
rebroadcast by matmul, with ln() applied on the PSUM chunks.
"""

import numpy as np

D0, D1F, D1C, D2 = 2048, 8192, 4096, 2048
NCORES = 8
R0 = D0 // NCORES  # theta0 rows per core (256)
C2 = D2 // NCORES  # theta1 cols per core (256)
T0T = R0 // 128  # theta0 SBUF tiles per core (2)
T1T = D1C // 128  # theta1 row tiles (32)
NCHUNK = D1F // 512  # 512-wide chunks of the fine axis (16)
PAD = 128  # zero slots appended to the S_a gather table

_cache = {}


def _build(cap, repeats=1, stage=99):
    import concourse.bacc as bacc
    import concourse.tile as tile
    from concourse import mybir

    f32 = mybir.dt.float32
    i16 = mybir.dt.int16
    Exp = mybir.ActivationFunctionType.Exp
    Ln = mybir.ActivationFunctionType.Ln
    X = mybir.AxisListType.X
    add = mybir.AluOpType.add

    nc = bacc.Bacc("TRN2", target_bir_lowering=False, debug=False, num_devices=NCORES)

    th0 = nc.dram_tensor("th0", [R0, D1F], f32, kind="ExternalInput").ap()
    th1 = nc.dram_tensor("th1", [D1C, C2], f32, kind="ExternalInput").ap()
    idxa = nc.dram_tensor("idxa", [128, D1F // 128], i16, kind="ExternalInput").ap()
    idxb = nc.dram_tensor("idxb", [128, D1C // 8 * cap // 16], i16,
                          kind="ExternalInput").ap()
    t0 = nc.dram_tensor("t0", [R0, D1F], f32, kind="ExternalOutput").ap()
    t1 = nc.dram_tensor("t1", [D1C, C2], f32, kind="ExternalOutput").ap()

    nseg = 512 * cap  # per-core gather positions for the segment sum

    with tile.TileContext(nc) as tc:
        with (
            tc.tile_pool(name="singles", bufs=1) as singles,
            tc.tile_pool(name="big", bufs=1) as big,
            tc.tile_pool(name="stg", bufs=3) as stg,
            tc.tile_pool(name="lnp", bufs=3) as lnp,
            tc.tile_pool(name="th1s", bufs=6) as th1s,
            tc.tile_pool(name="psum", bufs=4, space="PSUM") as psum,
            tc.tile_pool(name="pss", bufs=2, space="PSUM") as pss,
            tc.tile_pool(name="dram", bufs=1, space="DRAM") as dram,
        ):
            ones = singles.tile([128, 128], f32)
            nc.vector.memset(ones, 1.0)
            ones_col = singles.tile([1, 128], f32)
            nc.vector.memset(ones_col, 1.0)
            idxa_sb = singles.tile([128, D1F // 128], i16)
            nc.sync.dma_start(out=idxa_sb, in_=idxa)
            idxb_sb = singles.tile([128, D1C // 8 * cap // 16], i16)
            nc.sync.dma_start(out=idxb_sb, in_=idxb)

            for _rep in range(repeats):
                th0_sb = big.tile([128, T0T, D1F], f32, tag="th0")
                sa_tbl = big.tile([128, D1F + PAD], f32, tag="sa")
                sb_tbl = big.tile([128, D1C], f32, tag="sbt")
                segout = big.tile([128, nseg], f32, tag="seg")
                row_sb = singles.tile([1, D1F], f32, tag="row")
                mb_g = singles.tile([128, D1F // 8], f32, tag="mbg")
                sb_col = singles.tile([128, T1T], f32, tag="sbcol")
                sbl_col = singles.tile([128, T1T], f32, tag="sblcol")
                prod1 = singles.tile([128, T1T], f32, tag="prod1")
                red = singles.tile([128, 512], f32, tag="red")
                sa64 = singles.tile([128, 64], f32, tag="sa64")
                mb64 = singles.tile([128, 64], f32, tag="mb64")
                d0p = singles.tile([128, 1], f32, tag="d0p")
                recip0 = singles.tile([128, 1], f32, tag="rec0")
                recip1 = singles.tile([128, 1], f32, tag="rec1")
                pr_col = singles.tile([128, 1], f32, tag="prc")
                mg_lin = singles.tile([128, T1T], f32, tag="mgl")
                mg_col = singles.tile([128, T1T], f32, tag="mgc")

                cc_in = dram.tile([1, D1F + D1C], f32, tag="ccin")
                cc_out = dram.tile([1, D1F + D1C], f32, tag="ccout")
                mrow_d = dram.tile([1, D1F], f32, tag="mrow")
                msg_s = dram.tile([1, D1C], f32, tag="msgs")

                nc.vector.memset(sa_tbl[:, D1F:], 0.0)

                # ---- phase A: theta1 pass 1 -> S_b partials --------------
                for t in range(T1T):
                    t1_in = stg.tile([128, C2], f32, tag="t1a")
                    nc.sync.dma_start(out=t1_in, in_=th1[128 * t:128 * (t + 1), :])
                    e1 = stg.tile([128, C2], f32, tag="t1e")
                    nc.scalar.activation(e1, t1_in, Exp)
                    nc.vector.tensor_reduce(sb_col[:, t:t + 1], e1, axis=X, op=add)
                # natural [p, t] layout == sigma order sigma(c)=(c%128)*32+c//128
                nc.sync.dma_start(out=cc_in[0, D1F:], in_=sb_col)

                if stage < 2:
                    continue
                # ---- phase B: theta0 stream in -> S_a partials -----------
                for n in range(NCHUNK):
                    sl = slice(512 * n, 512 * (n + 1))
                    for t in range(T0T):
                        nc.sync.dma_start(
                            out=th0_sb[:, t, sl],
                            in_=th0[128 * t:128 * (t + 1), sl])
                    e0 = stg.tile([128, 512], f32, tag="e0")
                    e1b = stg.tile([128, 512], f32, tag="e1b")
                    nc.scalar.activation(e0, th0_sb[:, 0, sl], Exp)
                    nc.scalar.activation(e1b, th0_sb[:, 1, sl], Exp)
                    nc.vector.tensor_add(e0, e0, e1b)
                    ps = psum.tile([128, 512], f32, tag="ps")
                    nc.tensor.matmul(ps, ones, e0, start=True, stop=True)
                    # every PSUM row holds the same column sums; ship row 0
                    sa_row = stg.tile([1, 512], f32, tag="sarow")
                    nc.vector.tensor_copy(sa_row, ps[0:1, :])
                    nc.sync.dma_start(out=cc_in[0, sl], in_=sa_row)

                if stage < 3:
                    continue
                # ---- AllReduce -------------------------------------------
                nc.gpsimd.collective_compute(
                    "AllReduce",
                    mybir.AluOpType.add,
                    replica_groups=[list(range(NCORES))],
                    ins=[cc_in.opt()],
                    outs=[cc_out.opt()],
                )

                if stage < 4:
                    continue
                # ---- broadcast global tables via K=1 ones-matmuls --------
                # S_b first so the msg_ba gather can start early
                nc.sync.dma_start(out=row_sb[0:1, :D1C], in_=cc_out[0:1, D1F:])
                for n in range(D1C // 512):
                    sl = slice(512 * n, 512 * (n + 1))
                    ps = psum.tile([128, 512], f32, tag="ps")
                    nc.tensor.matmul(ps, ones_col, row_sb[0:1, sl],
                                     start=True, stop=True)
                    if n % 2 == 0:
                        nc.vector.tensor_copy(sb_tbl[:, sl], ps)
                    else:
                        nc.scalar.copy(sb_tbl[:, sl], ps)
                if stage >= 5:
                    # msg_ba gather, split 8 ways: core k's partition group
                    # gathers S_b[sigma(idx_a[j])] for j in [1024k, 1024k+1024)
                    nc.gpsimd.ap_gather(
                        out_ap=mb_g[:].rearrange("p (n d) -> p n d", d=1),
                        in_ap=sb_tbl[:].rearrange("p (n d) -> p n d", d=1),
                        idxs_ap=idxa_sb[:],
                        channels=128, num_elems=D1C, d=1, num_idxs=D1F // 8)
                nc.sync.dma_start(out=row_sb[0:1, :], in_=cc_out[0:1, :D1F])
                for n in range(NCHUNK):
                    sl = slice(512 * n, 512 * (n + 1))
                    ps = psum.tile([128, 512], f32, tag="ps")
                    nc.tensor.matmul(ps, ones_col, row_sb[0:1, sl],
                                     start=True, stop=True)
                    if n % 2 == 0:
                        nc.vector.tensor_copy(sa_tbl[:, sl], ps)
                    else:
                        nc.scalar.copy(sa_tbl[:, sl], ps)
                if stage >= 5:
                    # segment-sum gather: core k's group covers buckets
                    # [512k, 512k+512) x cap CSR slots into the S_a table
                    nc.gpsimd.ap_gather(
                        out_ap=segout[:].rearrange("p (n d) -> p n d", d=1),
                        in_ap=sa_tbl[:].rearrange("p (n d) -> p n d", d=1),
                        idxs_ap=idxb_sb[:],
                        channels=128, num_elems=D1F + PAD, d=1, num_idxs=nseg)
                # small c-layout vectors for the normalizer dots
                nc.sync.dma_start(
                    out=sbl_col,
                    in_=cc_out[0, D1F:].rearrange("(p t) -> p t", t=T1T))
                nc.sync.dma_start(
                    out=sa64,
                    in_=cc_out[0, :D1F].rearrange("(p t) -> p t", t=64))

                if stage < 6:
                    continue
                # ---- t0 path: dot0, rebroadcast msg_ba row, ln, add ------
                for k in range(8):
                    nc.sync.dma_start(
                        out=mrow_d[0, 1024 * k:1024 * (k + 1)],
                        in_=mb_g[16 * k:16 * k + 1, :])
                nc.sync.dma_start(
                    out=mb64, in_=mrow_d[0, :].rearrange("(p t) -> p t", t=64))
                nc.vector.tensor_mul(mb64, mb64, sa64)
                nc.vector.tensor_reduce(d0p, mb64, axis=X, op=add)
                d0ps = pss.tile([128, 1], f32, tag="d0ps")
                nc.tensor.matmul(d0ps, ones, d0p, start=True, stop=True)
                nc.vector.reciprocal(out=recip0, in_=d0ps)

                nc.sync.dma_start(out=row_sb[0:1, :], in_=mrow_d[0:1, :])
                for n in range(NCHUNK):
                    sl = slice(512 * n, 512 * (n + 1))
                    ps = psum.tile([128, 512], f32, tag="ps")
                    nc.tensor.matmul(ps, ones_col, row_sb[0:1, sl],
                                     start=True, stop=True)
                    lnc = lnp.tile([128, 512], f32, tag="lnc")
                    # msgba_adj = ln(S_b[idx_a[j]] / dot0), from PSUM
                    nc.scalar.activation(lnc, ps, Ln, scale=recip0[:, 0:1])
                    for t in range(T0T):
                        nc.vector.tensor_add(th0_sb[:, t, sl],
                                             th0_sb[:, t, sl], lnc)
                        nc.sync.dma_start(out=t0[128 * t:128 * (t + 1), sl],
                                          in_=th0_sb[:, t, sl])

                if stage < 7:
                    continue
                # ---- t1 path: segment sums, dot1, ln, add ----------------
                nc.vector.tensor_reduce(
                    red, segout[:].rearrange("p (b l) -> p b l", l=cap),
                    axis=X, op=add)
                for k in range(8):
                    nc.sync.dma_start(out=msg_s[0, 512 * k:512 * (k + 1)],
                                      in_=red[16 * k:16 * k + 1, :])
                # mg_lin[p, t] = seg[128t + p]
                nc.sync.dma_start(
                    out=mg_lin,
                    in_=msg_s[0, :].rearrange("(t p) -> p t", p=128))
                nc.vector.tensor_mul(prod1, mg_lin, sbl_col)
                nc.vector.tensor_reduce(pr_col, prod1, axis=X, op=add)
                prps = pss.tile([128, 1], f32, tag="prps")
                nc.tensor.matmul(prps, ones, pr_col, start=True, stop=True)
                nc.vector.reciprocal(out=recip1, in_=prps)
                # msgab_col[p, t] = ln(seg[128t+p] / dot1)
                nc.scalar.activation(mg_col, mg_lin, Ln, scale=recip1[:, 0:1])
                for t in range(T1T):
                    t1_io = th1s.tile([128, C2], f32, tag="t1b")
                    nc.sync.dma_start(out=t1_io, in_=th1[128 * t:128 * (t + 1), :])
                    nc.vector.tensor_scalar_add(t1_io, t1_io, mg_col[:, t:t + 1])
                    nc.sync.dma_start(out=t1[128 * t:128 * (t + 1), :], in_=t1_io)

    nc.compile()
    return nc


def _prep_inputs(theta0, theta1, idx_a, idx_b):
    """Host-side sharding + index-table construction (no float math)."""
    theta0 = np.ascontiguousarray(np.asarray(theta0, dtype=np.float32))
    theta1 = np.ascontiguousarray(np.asarray(theta1, dtype=np.float32))
    ia = np.asarray(idx_a).astype(np.int64)
    ib = np.asarray(idx_b).astype(np.int64)

    # sigma-permuted idx_a (S_b table is stored in sigma order), split 8 ways:
    # core k's group holds j in [1024k, 1024k+1024) wrapped over 16 partitions
    sig_ia = ((ia % 128) * 32 + ia // 128).astype(np.int16)
    idxa_w = np.ascontiguousarray(
        sig_ia.reshape(8, 64, 16).transpose(0, 2, 1).reshape(128, 64))

    # CSR with per-bucket capacity cap, padded with D1F (a zero slot)
    counts = np.bincount(ib, minlength=D1C)
    cap = int(max(2, counts.max()))
    order = np.argsort(ib, kind="stable")
    sc = ib[order]
    ranks = np.arange(D1F) - np.searchsorted(sc, sc, side="left")
    G = np.full((D1C, cap), D1F, dtype=np.int16)
    G[sc, ranks] = order.astype(np.int16)

    idxb_w = np.empty((128, 512 * cap // 16), dtype=np.int16)
    i = np.arange(512 * cap)
    for k in range(8):
        unwrapped = G[512 * k + i // cap, i % cap]
        idxb_w[16 * k:16 * (k + 1), :] = unwrapped.reshape(512 * cap // 16, 16).T

    in_maps = []
    for k in range(NCORES):
        in_maps.append({
            "th0": theta0[R0 * k:R0 * (k + 1), :],
            "th1": np.ascontiguousarray(theta1[:, C2 * k:C2 * (k + 1)]),
            "idxa": idxa_w,
            "idxb": idxb_w,
        })
    return cap, in_maps


def get_program(cap, repeats=1):
    key = ("prog", cap, repeats)
    if key not in _cache:
        _cache[key] = _build(cap, repeats)
    return _cache[key]


def kernel(theta0, theta1, idx_a, idx_b):
    from concourse.bass_utils import run_bass_kernel_spmd

    cap, in_maps = _prep_inputs(theta0, theta1, idx_a, idx_b)
    nc = get_program(cap)
    res = run_bass_kernel_spmd(nc, in_maps, list(range(NCORES)))
    t0 = np.concatenate([res.results[k]["t0"] for k in range(NCORES)], axis=0)
    t1 = np.concatenate([res.results[k]["t1"] for k in range(NCORES)], axis=1)
    return (t0, t1)


# revision 17
# speedup vs baseline: 3.8599x; 2.4717x over previous
"""Belief-propagation single-iteration kernel for 8 Trainium2 NeuronCores.

Problem (see reference):
    theta0: (2048, 8192) f32   clique A over (v0, v1_fine)
    theta1: (4096, 2048) f32   clique B over (v1_coarse, v2)
    idx_a, idx_b: (8192,) int64 maps fine->coarse

    marg_a = logsumexp(theta0, axis=0)                      # (8192,)
    msg_ab = segment_logsumexp(marg_a, idx_b, 4096)         # (4096,)
    marg_b = logsumexp(theta1, axis=1)                      # (4096,)
    msg_ba = marg_b[idx_a]                                  # (8192,)
    t0 = theta0 + msg_ba[None, :];  t0 -= logsumexp(t0)
    t1 = theta1 + msg_ab[:, None];  t1 -= logsumexp(t1)
    return (t0, t1)

Device algorithm works in the linear "sum-exp" domain:
    S_a[j] = sum_i exp(theta0[i,j])        (AllReduce over row shards)
    S_b[c] = sum_k exp(theta1[c,k])        (AllReduce over col shards)
    seg[c] = sum_{j: idx_b[j]=c} S_a[j]    (CSR-padded ap_gather + reduce)
    dot0 = sum_j S_a[j] * S_b[idx_a[j]]  = exp(Z0)
    dot1 = sum_c S_b[c] * seg[c]         = exp(Z1)
    t0 = theta0 + ln(S_b[idx_a[j]] / dot0)   (broadcast add over rows)
    t1 = theta1 + ln(seg[c] / dot1)          (broadcast add over cols)

Sharding: theta0 row-sharded (v0), theta1 column-sharded (v2); the small
marginal vectors are AllReduced and the message computation runs
replicated on every core.  Fully SPMD - no core-id dependence anywhere.

The S_b table is stored in a "sigma" permuted order sigma(c) =
(c % 128) * 32 + c // 128 so that the per-core [128, 32] partial-sum
tile DMAs to the collective buffer with a plain access pattern; idx_a
is remapped on the host to compensate.

Partition-broadcasts of the small tables use K=1 ones-matmuls on the PE
(a 0-stride broadcast DMA measures ~65 GB/s - far too slow).  The
msg_ba gather is split 8 ways across the GPSIMD cores (ap_gather costs
~20 ns per index per core), then reassembled through a DRAM row and
rebroadcast by matmul, with ln() applied on the PSUM chunks.
"""

import numpy as np

D0, D1F, D1C, D2 = 2048, 8192, 4096, 2048
NCORES = 8
R0 = D0 // NCORES  # theta0 rows per core (256)
C2 = D2 // NCORES  # theta1 cols per core (256)
T0T = R0 // 128  # theta0 SBUF tiles per core (2)
T1T = D1C // 128  # theta1 row tiles (32)
NCHUNK = D1F // 512  # 512-wide chunks of the fine axis (16)
PAD = 128  # zero slots appended to the S_a gather table

_cache = {}


def _build(cap, repeats=1, stage=99):
    import concourse.bacc as bacc
    import concourse.tile as tile
    from concourse import mybir

    f32 = mybir.dt.float32
    i16 = mybir.dt.int16
    Exp = mybir.ActivationFunctionType.Exp
    Ln = mybir.ActivationFunctionType.Ln
    X = mybir.AxisListType.X
    add = mybir.AluOpType.add

    nc = bacc.Bacc("TRN2", target_bir_lowering=False, debug=False, num_devices=NCORES)

    th0 = nc.dram_tensor("th0", [R0, D1F], f32, kind="ExternalInput").ap()
    th1 = nc.dram_tensor("th1", [D1C, C2], f32, kind="ExternalInput").ap()
    idxa = nc.dram_tensor("idxa", [128, D1F // 128], i16, kind="ExternalInput").ap()
    lob = nc.dram_tensor("lob", [128, 64], f32, kind="ExternalInput").ap()
    hib = nc.dram_tensor("hib", [128, 64], f32, kind="ExternalInput").ap()
    iota = nc.dram_tensor("iota", [128, 128], f32, kind="ExternalInput").ap()
    t0 = nc.dram_tensor("t0", [R0, D1F], f32, kind="ExternalOutput").ap()
    t1 = nc.dram_tensor("t1", [D1C, C2], f32, kind="ExternalOutput").ap()

    with tile.TileContext(nc) as tc:
        with (
            tc.tile_pool(name="singles", bufs=1) as singles,
            tc.tile_pool(name="big", bufs=1) as big,
            tc.tile_pool(name="stg", bufs=3) as stg,
            tc.tile_pool(name="lnp", bufs=3) as lnp,
            tc.tile_pool(name="th1s", bufs=6) as th1s,
            tc.tile_pool(name="psum", bufs=4, space="PSUM") as psum,
            tc.tile_pool(name="pss", bufs=1, space="PSUM") as pss,
            tc.tile_pool(name="dram", bufs=1, space="DRAM") as dram,
        ):
            ones = singles.tile([128, 128], f32)
            nc.vector.memset(ones, 1.0)
            ones_col = singles.tile([1, 128], f32)
            nc.vector.memset(ones_col, 1.0)
            idxa_sb = singles.tile([128, D1F // 128], i16)
            nc.sync.dma_start(out=idxa_sb, in_=idxa)
            lob_sb = singles.tile([128, 64], f32)
            nc.sync.dma_start(out=lob_sb, in_=lob)
            hib_sb = singles.tile([128, 64], f32)
            nc.sync.dma_start(out=hib_sb, in_=hib)
            iota_sb = singles.tile([128, 128], f32)
            nc.sync.dma_start(out=iota_sb, in_=iota)

            for _rep in range(repeats):
                th0_sb = big.tile([128, T0T, D1F], f32, tag="th0")
                sb_tbl = big.tile([128, D1C], f32, tag="sbt")
                row_sb = singles.tile([1, D1F], f32, tag="row")
                mb_g = singles.tile([128, D1F // 8], f32, tag="mbg")
                sb_col = singles.tile([128, T1T], f32, tag="sbcol")
                sbl_col = singles.tile([128, T1T], f32, tag="sblcol")
                prod1 = singles.tile([128, T1T], f32, tag="prod1")
                sa64 = singles.tile([128, 64], f32, tag="sa64")
                mb64 = singles.tile([128, 64], f32, tag="mb64")
                d0p = singles.tile([128, 1], f32, tag="d0p")
                recip0 = singles.tile([128, 1], f32, tag="rec0")
                recip1 = singles.tile([128, 1], f32, tag="rec1")
                pr_col = singles.tile([128, 1], f32, tag="prc")
                mg_col = singles.tile([128, T1T], f32, tag="mgc")

                sa_jt = singles.tile([128, 64], f32, tag="sajt")

                cc_in = dram.tile([1, D1F + D1C], f32, tag="ccin")
                cc_out = dram.tile([1, D1F + D1C], f32, tag="ccout")
                mrow_d = dram.tile([1, D1F], f32, tag="mrow")

                # ---- phase A: theta1 pass 1 -> S_b partials --------------
                for t in range(T1T):
                    t1_in = stg.tile([128, C2], f32, tag="t1a")
                    nc.sync.dma_start(out=t1_in, in_=th1[128 * t:128 * (t + 1), :])
                    e1 = stg.tile([128, C2], f32, tag="t1e")
                    nc.scalar.activation(e1, t1_in, Exp)
                    nc.vector.tensor_reduce(sb_col[:, t:t + 1], e1, axis=X, op=add)
                # natural [p, t] layout == sigma order sigma(c)=(c%128)*32+c//128
                nc.sync.dma_start(out=cc_in[0, D1F:], in_=sb_col)

                if stage < 2:
                    continue
                # ---- phase B: theta0 stream in -> S_a partials -----------
                for n in range(NCHUNK):
                    sl = slice(512 * n, 512 * (n + 1))
                    for t in range(T0T):
                        nc.sync.dma_start(
                            out=th0_sb[:, t, sl],
                            in_=th0[128 * t:128 * (t + 1), sl])
                    e0 = stg.tile([128, 512], f32, tag="e0")
                    e1b = stg.tile([128, 512], f32, tag="e1b")
                    nc.scalar.activation(e0, th0_sb[:, 0, sl], Exp)
                    nc.scalar.activation(e1b, th0_sb[:, 1, sl], Exp)
                    nc.vector.tensor_add(e0, e0, e1b)
                    ps = psum.tile([128, 512], f32, tag="ps")
                    nc.tensor.matmul(ps, ones, e0, start=True, stop=True)
                    # every PSUM row holds the same column sums; ship row 0
                    sa_row = stg.tile([1, 512], f32, tag="sarow")
                    nc.vector.tensor_copy(sa_row, ps[0:1, :])
                    nc.sync.dma_start(out=cc_in[0, sl], in_=sa_row)

                if stage < 3:
                    continue
                # ---- AllReduce -------------------------------------------
                nc.gpsimd.collective_compute(
                    "AllReduce",
                    mybir.AluOpType.add,
                    replica_groups=[list(range(NCORES))],
                    ins=[cc_in.opt()],
                    outs=[cc_out.opt()],
                )

                if stage < 4:
                    continue
                # ---- broadcast global tables via K=1 ones-matmuls --------
                # S_b first so the msg_ba gather can start early
                nc.sync.dma_start(out=row_sb[0:1, :D1C], in_=cc_out[0:1, D1F:])
                for n in range(D1C // 512):
                    sl = slice(512 * n, 512 * (n + 1))
                    ps = psum.tile([128, 512], f32, tag="ps")
                    nc.tensor.matmul(ps, ones_col, row_sb[0:1, sl],
                                     start=True, stop=True)
                    if n % 2 == 0:
                        nc.vector.tensor_copy(sb_tbl[:, sl], ps)
                    else:
                        nc.scalar.copy(sb_tbl[:, sl], ps)
                if stage >= 5:
                    # msg_ba gather, split 8 ways: core k's partition group
                    # gathers S_b[sigma(idx_a[j])] for j in [1024k, 1024k+1024)
                    nc.gpsimd.ap_gather(
                        out_ap=mb_g[:].rearrange("p (n d) -> p n d", d=1),
                        in_ap=sb_tbl[:].rearrange("p (n d) -> p n d", d=1),
                        idxs_ap=idxa_sb[:],
                        channels=128, num_elems=D1C, d=1, num_idxs=D1F // 8)
                # small c-layout vectors for the normalizer dots
                nc.sync.dma_start(
                    out=sbl_col,
                    in_=cc_out[0, D1F:].rearrange("(p t) -> p t", t=T1T))
                nc.sync.dma_start(
                    out=sa64,
                    in_=cc_out[0, :D1F].rearrange("(p t) -> p t", t=64))
                # S_a in (j % 128, j // 128) layout for the one-hot seg sum
                nc.sync.dma_start(
                    out=sa_jt,
                    in_=cc_out[0, :D1F].rearrange("(t p) -> p t", p=128))
                if stage >= 5:
                    # segment sums on the PE: psum_seg[p, t] = seg[128t + p]
                    #   = sum_j 1[idx_b[j] % 128 = p] * 1[idx_b[j]//128 = t]
                    #         * S_a[j]
                    psum_seg = pss.tile([128, T1T], f32, tag="segps")
                    for jt in range(64):
                        loh = stg.tile([128, 128], f32, tag="loh")
                        nc.vector.tensor_scalar(
                            loh, iota_sb, lob_sb[:, jt:jt + 1], None,
                            op0=mybir.AluOpType.is_equal)
                        him = stg.tile([128, T1T], f32, tag="him")
                        nc.vector.tensor_scalar(
                            him, iota_sb[:, :T1T], hib_sb[:, jt:jt + 1],
                            sa_jt[:, jt:jt + 1],
                            op0=mybir.AluOpType.is_equal,
                            op1=mybir.AluOpType.mult)
                        nc.tensor.matmul(psum_seg, loh, him,
                                         start=(jt == 0), stop=(jt == 63))

                if stage < 6:
                    continue
                # ---- t0 path: dot0, rebroadcast msg_ba row, ln, add ------
                for k in range(8):
                    nc.sync.dma_start(
                        out=mrow_d[0, 1024 * k:1024 * (k + 1)],
                        in_=mb_g[16 * k:16 * k + 1, :])
                nc.sync.dma_start(
                    out=mb64, in_=mrow_d[0, :].rearrange("(p t) -> p t", t=64))
                nc.vector.tensor_mul(mb64, mb64, sa64)
                nc.vector.tensor_reduce(d0p, mb64, axis=X, op=add)
                d0ps = pss.tile([128, 1], f32, tag="d0ps")
                nc.tensor.matmul(d0ps, ones, d0p, start=True, stop=True)
                nc.vector.reciprocal(out=recip0, in_=d0ps)

                nc.sync.dma_start(out=row_sb[0:1, :], in_=mrow_d[0:1, :])
                for n in range(NCHUNK):
                    sl = slice(512 * n, 512 * (n + 1))
                    ps = psum.tile([128, 512], f32, tag="ps")
                    nc.tensor.matmul(ps, ones_col, row_sb[0:1, sl],
                                     start=True, stop=True)
                    lnc = lnp.tile([128, 512], f32, tag="lnc")
                    # msgba_adj = ln(S_b[idx_a[j]] / dot0), from PSUM
                    nc.scalar.activation(lnc, ps, Ln, scale=recip0[:, 0:1])
                    for t in range(T0T):
                        nc.vector.tensor_add(th0_sb[:, t, sl],
                                             th0_sb[:, t, sl], lnc)
                        nc.sync.dma_start(out=t0[128 * t:128 * (t + 1), sl],
                                          in_=th0_sb[:, t, sl])

                if stage < 7:
                    continue
                # ---- t1 path: dot1, ln, add ------------------------------
                nc.vector.tensor_mul(prod1, psum_seg, sbl_col)
                nc.vector.tensor_reduce(pr_col, prod1, axis=X, op=add)
                prps = pss.tile([128, 1], f32, tag="prps")
                nc.tensor.matmul(prps, ones, pr_col, start=True, stop=True)
                nc.vector.reciprocal(out=recip1, in_=prps)
                # msgab_col[p, t] = ln(seg[128t+p] / dot1)
                nc.scalar.activation(mg_col, psum_seg, Ln, scale=recip1[:, 0:1])
                for t in range(T1T):
                    t1_io = th1s.tile([128, C2], f32, tag="t1b")
                    nc.sync.dma_start(out=t1_io, in_=th1[128 * t:128 * (t + 1), :])
                    nc.vector.tensor_scalar_add(t1_io, t1_io, mg_col[:, t:t + 1])
                    nc.sync.dma_start(out=t1[128 * t:128 * (t + 1), :], in_=t1_io)

    nc.compile()
    return nc


def _prep_inputs(theta0, theta1, idx_a, idx_b):
    """Host-side sharding + index-table construction (no float math)."""
    theta0 = np.ascontiguousarray(np.asarray(theta0, dtype=np.float32))
    theta1 = np.ascontiguousarray(np.asarray(theta1, dtype=np.float32))
    ia = np.asarray(idx_a).astype(np.int64)
    ib = np.asarray(idx_b).astype(np.int64)

    # sigma-permuted idx_a (S_b table is stored in sigma order), split 8 ways:
    # core k's group holds j in [1024k, 1024k+1024) wrapped over 16 partitions
    sig_ia = ((ia % 128) * 32 + ia // 128).astype(np.int16)
    idxa_w = np.ascontiguousarray(
        sig_ia.reshape(8, 64, 16).transpose(0, 2, 1).reshape(128, 64))

    # idx_b lo/hi tables in (j % 128, j // 128) layout for the one-hot
    # segment sum: seg lands as [p = c % 128, t = c // 128]
    ibm = ib.reshape(64, 128).T
    lob_w = np.ascontiguousarray((ibm % 128).astype(np.float32))
    hib_w = np.ascontiguousarray((ibm // 128).astype(np.float32))
    iota_w = np.tile(np.arange(128, dtype=np.float32)[None, :], (128, 1))

    in_maps = []
    for k in range(NCORES):
        in_maps.append({
            "th0": theta0[R0 * k:R0 * (k + 1), :],
            "th1": np.ascontiguousarray(theta1[:, C2 * k:C2 * (k + 1)]),
            "idxa": idxa_w,
            "lob": lob_w,
            "hib": hib_w,
            "iota": iota_w,
        })
    return 0, in_maps


def get_program(cap, repeats=1):
    key = ("prog", cap, repeats)
    if key not in _cache:
        _cache[key] = _build(cap, repeats)
    return _cache[key]


def kernel(theta0, theta1, idx_a, idx_b):
    from concourse.bass_utils import run_bass_kernel_spmd

    cap, in_maps = _prep_inputs(theta0, theta1, idx_a, idx_b)
    nc = get_program(cap)
    res = run_bass_kernel_spmd(nc, in_maps, list(range(NCORES)))
    t0 = np.concatenate([res.results[k]["t0"] for k in range(NCORES)], axis=0)
    t1 = np.concatenate([res.results[k]["t1"] for k in range(NCORES)], axis=1)
    return (t0, t1)
